# revision 12
# baseline (speedup 1.0000x reference)
"""Trainium2 Bass kernel for nn_DPLoss (histogram_binning).

Data-parallel over batch: 2 batches per core on 8 cores. Per batch b,
class c (C=4, only c>=1 contribute) the device computes
  A_c = sum_p [t==c] * x_c          (masked logit sum)
  B_c = sum_p [t==c] * L            (masked lse sum), L = log(sum_c exp(x_c))
  H_c = sum_p [x_c == max_c' x_c']  (pred histogram, fp16 compare)
Host combines:
  D_c = A_c - B_c                   (CE numerator; dp_loss = -D/HW)
  loss = sum_{b,c>=1} w[b,c] * (-D_c - H_c) / (H*W)
  w = sigmoid(bw); w /= w.mean(axis=0); w /= (1+e)

x and t are cast to fp16 during the SWDGE loads (HBM traffic unchanged);
x planes are loaded in class PAIRS (x01, x23) so each stage costs 3 Q7
descriptor emissions instead of 5 and the DVE waits on fewer semaphores.
Engine split per stage:
  ACT: E_c = exp(x_c) x4, L = ln(S) per chunk; final copy+accum_out
       column-reduces the selector accumulator so 'o' is [128,4].
  PE : S = sum_c E_c (identity matmuls into psum);
       A/H selector reductions column-tiled 4-wide: plane (b,q) reduces
       into psum partition 32*(q%4) + (b*6+q) of a [128,512] accumulator,
       so up to 4 reductions run concurrently in separate PE column groups;
       B_c trace blocks: lhsT = L chunk (shared by 3 classes), rhs = mask
       chunk -> per-class psum [128,128]; host takes the trace.
  DVE: 3 TS masks (4x), 3 TT max tree (2x), 3 TT eq (2x), 3 TT mask*x (2x)
       -- no back-half DVE ops, so the DVE queue never stalls on ACT/PE.
Batch 0's first stage is narrow (512) so DVE/ACT start early; batch 1
ends with a 256 stage to shorten the closing dependency chain. Each
batch's B trace blocks are drained and DMA'd out as soon as the batch
closes (batch 0's half overlaps the stream).
"""

import numpy as np

_B, _C, _H, _W = 16, 4, 768, 768
_HW = _H * _W            # 589824
_NCORES = 8
_NB = _B // _NCORES      # 2 batches per core
_P = 128
_FREE = _HW // _P        # 4608
_SW = 2176               # max stage width (free-dim columns per tile)
_CHUNK = 512             # psum / matmul chunk
_NQ = 6                  # A1,A2,A3,H1,H2,H3
_NSEL = _NB * _NQ        # 12 selector blocks of 32 columns

_nc_cache = None


def _patch_act_tables():
    """Force a single activation table set (has Exp, Ln, Copy) so the
    compiler doesn't thrash table loads between Exp and Ln sets."""
    import concourse.bacc as bacc_mod
    import concourse.hw_specs as hw_specs

    if getattr(bacc_mod, "_act_tables_patched", False):
        return
    orig = hw_specs.get_activation_tables

    def patched(module_arch):
        t = orig(module_arch)
        keep = "natural_log_exp_and_others"
        return {k: (v if k == keep else set()) for k, v in t.items()}

    bacc_mod.get_activation_tables = patched
    bacc_mod._act_tables_patched = True


def _chunks(w):
    out = []
    off = 0
    while off < w:
        cw = min(_CHUNK, w - off)
        out.append((off, cw))
        off += cw
    return out


def _build():
    import concourse.tile as tile
    from concourse import bacc, mybir

    _patch_act_tables()

    f32 = mybir.dt.float32
    f16 = mybir.dt.float16
    i32 = mybir.dt.int32
    AF = mybir.ActivationFunctionType
    OP = mybir.AluOpType

    nc = bacc.Bacc(
        "TRN2",
        target_bir_lowering=False,
        debug=False,
        enable_asserts=False,
        num_devices=_NCORES,
    )
    x = nc.dram_tensor("x", [_NB, _C, _P, _FREE], f32, kind="ExternalInput").ap()
    # t is repacked to uint8 on the host (values 0..3, lossless) so the
    # target load costs 1/4 of the original int32 HBM traffic.
    t = nc.dram_tensor("t", [_NB, _P, _FREE], mybir.dt.uint8,
                       kind="ExternalInput").ap()
    cst = nc.dram_tensor("c", [_P, 128 + _NSEL * 32], f16,
                         kind="ExternalInput").ap()
    outb = nc.dram_tensor("ob", [_P, _NB * 3 * 128 + 4], f32,
                          kind="ExternalOutput").ap()

    # group of plane q is q % 4; the last plane of each group closes it
    maxq = {0: 4, 1: 5, 2: 2, 3: 3}

    # stage splits: batch 0 starts narrow (early DVE/ACT start), batch 1
    # ends narrow (short closing chain)
    stages_of = {
        0: [(0, 256), (256, 2176), (2432, 2176)],
        _NB - 1: [(0, 2176), (2176, 2176), (4352, 256)],
    }

    with tile.TileContext(nc) as tc:
        with (
            tc.tile_pool(name="const", bufs=1) as constp,
            tc.tile_pool(name="xin", bufs=3) as xin,
            tc.tile_pool(name="tin", bufs=2) as tin,
            tc.tile_pool(name="ework", bufs=2) as ework,
            tc.tile_pool(name="work", bufs=2) as work,
            tc.tile_pool(name="prodp", bufs=2) as prodp,
            tc.tile_pool(name="outp", bufs=1) as outp,
            tc.tile_pool(name="ps", bufs=3, space="PSUM") as ps,
            tc.tile_pool(name="psacc", bufs=1, space="PSUM") as psacc,
            tc.tile_pool(name="pstr", bufs=1, space="PSUM") as pstr,
        ):
            # consts (one DMA): identity [128,128] then 12 selector blocks of
            # 32 cols — block r = b*6+q has ones in LOCAL column r only, so a
            # col-tiled matmul lands the column-sums in psum partition
            # 32*(q%4) + r and adds zeros to the rest of that group.
            cstt = constp.tile([_P, 128 + _NSEL * 32], f16)
            nc.sync.dma_start(cstt[:], cst)
            ident = cstt[:, 0:128]
            accps = psacc.tile([_P, _CHUNK], f32)
            btile = outp.tile([_P, _NB * 3 * 128 + 4], f32, tag="btile")
            osc = outp.tile([_P, _CHUNK], f16, tag="osc")

            # deferred per-stage back-half (S-sum, ln, B-traces, selector
            # reductions): emitted one iteration later so each engine's
            # in-order queue sees the next stage's front-half (exp, masks,
            # max, eq) before this stage's latency-chained ops.
            pend = {}

            def flush():
                if not pend:
                    return
                et_, mk_, tq_, btr_, b_, first_, last_, w_ = (
                    pend["et"], pend["mk"], pend["tq"], pend["btr"],
                    pend["b"], pend["first"], pend["last"], pend["w"])
                L = work.tile([_P, _SW], f16, tag="L", name="L")
                for chs_off, cw in _chunks(w_):
                    chs = slice(chs_off, chs_off + cw)
                    S = ps.tile([_P, _CHUNK], f32, tag="S", name="S")
                    for c in range(_C):
                        nc.tensor.matmul(
                            S[:, 0:cw], ident,
                            et_[:, c * _SW + chs_off: c * _SW + chs_off + cw],
                            start=(c == 0), stop=(c == _C - 1),
                        )
                    nc.scalar.activation(L[:, chs], S[:, 0:cw], AF.Ln)
                for k in range(w_ // 128):
                    ks = slice(k * 128, (k + 1) * 128)
                    for i in range(3):
                        nc.tensor.matmul(
                            btr_[i][:], L[:, ks], mk_[i][:, ks],
                            start=(first_ and k == 0),
                            stop=(last_ and k == w_ // 128 - 1),
                            skip_group_check=True,
                        )
                for q, tq in enumerate(tq_):
                    r = b_ * _NQ + q
                    g = q % 4
                    sel = cstt[:, 128 + r * 32: 128 + (r + 1) * 32]
                    for chs_off, cw in _chunks(w_):
                        g_first = b_ == 0 and first_ and q == g and chs_off == 0
                        g_last = (b_ == _NB - 1 and last_
                                  and q == maxq[g] and chs_off + cw == w_)
                        nc.tensor.matmul(
                            accps[32 * g: 32 * (g + 1), 0:cw], sel,
                            tq[:, chs_off:chs_off + cw],
                            start=g_first, stop=g_last,
                            skip_group_check=True,
                            tile_position=(0, 32 * g),
                        )
                if last_:
                    # drain this batch's trace blocks to sbuf + DMA them out
                    for i in range(3):
                        off = (b_ * 3 + i) * 128
                        nc.scalar.copy(btile[:, off:off + 128], btr_[i][:])
                    if b_ == 0:
                        nc.sync.dma_start(outb[:, 0:384], btile[:, 0:384])
                pend.clear()

            for b in range(_NB):
                tb = tin.tile([_P, _FREE], f16, tag="tb")
                btr = [pstr.tile([_P, 128], f32, tag=f"btr{c}", name=f"btr{c}")
                       for c in (1, 2, 3)]

                stages = stages_of.get(b, [(0, 2176), (2176, 2176), (4352, 256)])

                for si, (off, w) in enumerate(stages):
                    sl = slice(off, off + w)
                    first = si == 0
                    last = si == len(stages) - 1

                    # paired cast DMAs (one Q7 emission + one semaphore per
                    # class pair). The very first stage loads t first so the
                    # DVE's mask ops can start as early as possible; later
                    # stages order x01, t, x23 so the max tree's first input
                    # pair lands earliest.
                    t_early = b == 0 and first
                    if t_early:
                        nc.gpsimd.dma_start(tb[:, sl], t[b, :, sl])
                    x01 = xin.tile([_P, 2 * w], f16, tag="x01",
                                   padded_shape=[_P, 2 * _SW])
                    nc.gpsimd.dma_start(
                        x01[:], x[b, 0:2, :, sl].rearrange("c p w -> p c w"))
                    if not t_early:
                        nc.gpsimd.dma_start(tb[:, sl], t[b, :, sl])
                    x23 = xin.tile([_P, 2 * w], f16, tag="x23",
                                   padded_shape=[_P, 2 * _SW])
                    nc.gpsimd.dma_start(
                        x23[:], x[b, 2:4, :, sl].rearrange("c p w -> p c w"))
                    xc = [x01[:, 0:w], x01[:, w:2 * w],
                          x23[:, 0:w], x23[:, w:2 * w]]

                    # --- ACT: E_c = exp(x_c) into one 2D tile ---
                    et = ework.tile([_P, _C * _SW], f16, tag="E")
                    for c in range(_C):
                        nc.scalar.activation(
                            et[:, c * _SW: c * _SW + w], xc[c], AF.Exp)

                    # --- DVE in data-arrival order: masks right after t,
                    # max halves as the class pairs land ---
                    def emit_masks():
                        mk = []
                        for i, c in enumerate((1, 2, 3)):
                            mc = work.tile([_P, _SW], f16, tag=f"m{c}",
                                           name=f"m{c}")
                            nc.vector.tensor_scalar(
                                mc[:, :w], tb[:, sl], float(c), None,
                                op0=OP.is_equal)
                            mk.append(mc[:, :w])
                        return mk

                    if t_early:
                        mk = emit_masks()
                    m01 = work.tile([_P, _SW], f16, tag="m01")
                    nc.vector.tensor_tensor(m01[:, :w], xc[0], xc[1], op=OP.max)
                    if not t_early:
                        mk = emit_masks()
                    tiles_q = []
                    p1 = prodp.tile([_P, _SW], f16, tag="p1")
                    nc.vector.tensor_tensor(p1[:, :w], mk[0][:], xc[1], op=OP.mult)
                    m23 = work.tile([_P, _SW], f16, tag="m23")
                    nc.vector.tensor_tensor(m23[:, :w], xc[2], xc[3], op=OP.max)
                    p2 = prodp.tile([_P, _SW], f16, tag="p2")
                    nc.vector.tensor_tensor(p2[:, :w], mk[1][:], xc[2], op=OP.mult)
                    p3 = prodp.tile([_P, _SW], f16, tag="p3")
                    nc.vector.tensor_tensor(p3[:, :w], mk[2][:], xc[3], op=OP.mult)
                    tiles_q += [p1, p2, p3]  # q = 0,1,2 -> A_c
                    M = work.tile([_P, _SW], f16, tag="M")
                    nc.vector.tensor_tensor(M[:, :w], m01[:, :w], m23[:, :w],
                                            op=OP.max)

                    # --- DVE eq planes (TT 2x), written in-place into the
                    # dead max-tree tiles (m01/m23 are dead once M exists; M
                    # dies at the last eq). Same-engine in-order, no hazards.
                    for c, dst in zip((1, 2, 3), (m01, m23, M)):
                        nc.vector.tensor_tensor(dst[:, :w], xc[c], M[:, :w],
                                                op=OP.is_equal)
                        tiles_q.append(dst)  # q = 3,4,5 -> H_c

                    # back-half of the PREVIOUS stage, then defer this one
                    flush()
                    pend.update(et=et, mk=mk, tq=tiles_q, btr=btr, b=b,
                                first=first, last=last, w=w)

            flush()

            # column-reduce the A/H accumulator on the way out: the ACT
            # copy's accum_out drops the per-partition sums into btile's
            # tail columns, so one DMA ships batch 1's traces + the sums.
            nc.scalar.activation(osc[:], accps[:], AF.Copy,
                                 accum_out=btile[:, 768:769])
            nc.sync.dma_start(outb[:, 384:772], btile[:, 384:772])
    nc.compile()
    return nc


def _get_nc():
    global _nc_cache
    if _nc_cache is None:
        _nc_cache = _build()
    return _nc_cache


def _make_consts():
    import ml_dtypes

    cst = np.zeros((_P, 128 + _NSEL * 32), np.float32)
    cst[:, :128] = np.eye(128, dtype=np.float32)
    for r in range(_NSEL):
        cst[:, 128 + r * 32 + r] = 1.0
    return cst.astype(ml_dtypes.float16 if hasattr(ml_dtypes, "float16") else np.float16)


def _make_in_maps(net_output, target):
    net_output = np.ascontiguousarray(net_output, dtype=np.float32)
    # lossless repack: class ids 0..3 fit in uint8, quartering t's HBM cost
    target = np.asarray(target).astype(np.uint8)
    cst = np.ascontiguousarray(_make_consts())
    in_maps = []
    for k in range(_NCORES):
        xs = net_output[_NB * k: _NB * (k + 1)].reshape(_NB, _C, _P, _FREE)
        ts = target[_NB * k: _NB * (k + 1), 0].reshape(_NB, _P, _FREE)
        in_maps.append({"x": np.ascontiguousarray(xs), "t": np.ascontiguousarray(ts),
                        "c": cst})
    return in_maps


def _combine(results, bare_weight):
    # results: per core:
    #   'o'  [P, 4]: col 0 holds the column-sums of the A/H accumulator;
    #        plane (b,q) sum lives in partition 32*(q%4) + b*6 + q
    #   'ob' [P, NB*3*128]: B trace blocks per (batch, class)
    D = np.zeros((_B, _C), np.float64)
    Hc = np.zeros((_B, _C), np.float64)
    for k, r in enumerate(results):
        ob = r["ob"].astype(np.float64)
        o = ob[:, 768]
        for bb in range(_NB):
            gb = _NB * k + bb
            A = np.array([o[32 * (q % 4) + bb * _NQ + q] for q in (0, 1, 2)])
            Hq = np.array([o[32 * (q % 4) + bb * _NQ + q] for q in (3, 4, 5)])
            Bm = np.array([
                np.trace(ob[:, (bb * 3 + i) * 128:(bb * 3 + i + 1) * 128])
                for i in range(3)
            ])
            D[gb, 1:4] = A - Bm
            Hc[gb, 1:4] = Hq
    bw = bare_weight.astype(np.float64)
    sig = 1.0 / (1.0 + np.exp(-bw))
    w = sig / sig.mean(axis=0, keepdims=True)
    w = w / (1.0 + np.e)  # fixed_w for classes >= 1
    loss = (w[:, 1:] * (-D[:, 1:] - Hc[:, 1:])).sum() / _HW
    return np.float32(loss)


def _enable_jax_cache():
    # Persistent XLA-executable cache: the compiled NEFF is embedded in the
    # executable, so warm processes skip the ~3 min walrus compile entirely.
    try:
        import jax

        jax.config.update("jax_compilation_cache_dir", "/tmp/jax_bass_cache")
        jax.config.update("jax_persistent_cache_min_compile_time_secs", 1.0)
    except Exception:
        pass


def _run(net_output, target, bare_weight, **spmd_kwargs):
    from concourse.bass_utils import run_bass_kernel_spmd

    _enable_jax_cache()
    nc = _get_nc()
    in_maps = _make_in_maps(net_output, target)
    res = run_bass_kernel_spmd(nc, in_maps, core_ids=list(range(_NCORES)), **spmd_kwargs)
    return _combine(res.results, np.asarray(bare_weight)), res


def kernel(net_output, target, bare_weight):
    loss, _ = _run(np.asarray(net_output), np.asarray(target), np.asarray(bare_weight))
    return loss
